# revision 22
# baseline (speedup 1.0000x reference)
"""Trainium2 Bass kernel for nn_LRSVConv (low-rank spatially-varying conv).

Computes, for full inputs
    x            [8, 32, 256, 256]  f32
    conv_w       [192, 32, 3, 3]    f32   (192 = RANK(3) * C_OUT(64))
    kernel_weight[2, 256, 256]      f32
the reference:
    y   = conv2d(x, conv_w, stride 1, pad 1)      # [8, 192, 256, 256]
    y   = y.reshape(8, 3, 64, 256, 256)
    out = y[:,0] + kw[0]*y[:,1] + kw[1]*y[:,2]    # [8, 64, 256, 256]

Sharding: spatial (H) bands of 32 output rows across the 8 cores, all
batches per core, so the per-pixel blend weights are loaded once per core.

Per core, per supertile (4 output rows = 2 blocks q of 512 px):
  - conv matmuls in bf16 (FWL-able weight loads; rel err 3.4e-3 vs the
    2e-2 gate), K=96 (3 kh x 32 c_in), kw via free-dim shifts of the
    imcol tile (kh replicas pre-built host-side so each batch is ONE
    dma_start):
      per kw: bc[q0] (M=128: ranks 1|2), bc[q1], and a col-tiled
      concurrent pair a2[0:64] / a2[64:128] (M=64: rank 0, blocks q0/q1)
  - blend: m = bc * svb on DVE, then a col-tiled pair of identity
    matmuls folds m's two 64-row halves into a2 (which holds rank-0)
    -> out = y0 + sv1*y1 + sv2*y2 in PSUM; ACT evacuates to SBUF.
  - blend stage runs one supertile behind the conv stage so the PE
    never waits for DVE.
  - DMA: per-batch imcol prefetched mid-previous-batch; consts merged
    into single dma_starts (the HWDGE sequencer costs ~700ns per
    dma_start, and loads FIFO behind each other).
"""

import numpy as np
import ml_dtypes

B, C_IN, C_OUT, RANK, IMG = 8, 32, 64, 3, 256
N_CORES = 8
BAND = IMG // N_CORES          # 32 output rows per core
WP = IMG + 2                   # padded width 258
ROWS_IN = BAND + 2             # input rows needed per band (with halo)
SUPER = 8                      # supertiles per (batch, band): 4 rows each
SROWS = BAND // SUPER          # 4 image rows per supertile
NBLK = 512                     # pixels per matmul block (2 image rows)
IMW = BAND * WP                # imcol free size per kh block (8256)

_F32 = np.float32
_BF16 = ml_dtypes.bfloat16

# packed const layout (free-dim offsets in the [128, 640] wpack tile)
_OFF_WTBC = 0            # [0:96, kw*128 : kw*128+128], kw = 0..2
_OFF_WTA = 384           # [0:96, 384+kw*64 : +64]
_OFF_ID = 576            # [0:128, 576:640] = [I64; I64]
_WPACK_W = 640


def _build_bass():
    import concourse.mybir as mybir
    import concourse.tile as tile
    from concourse import bacc

    f32 = mybir.dt.float32
    bf16 = mybir.dt.bfloat16
    nc = bacc.Bacc("TRN2", target_bir_lowering=False, debug=False)

    # xs: host-prepped imcol source, partition dim (kh, c) pre-replicated
    xs_t = nc.dram_tensor("xs", (B, 96, IMW), bf16, kind="ExternalInput")
    wpack_t = nc.dram_tensor("wpack", (128, _WPACK_W), bf16, kind="ExternalInput")
    # svb rows 0:64 = sv1 (bcast over c), rows 64:128 = sv2; cols = (t, q, j)
    svb_t = nc.dram_tensor("svb", (128, SUPER * 2 * NBLK), bf16, kind="ExternalInput")
    out_t = nc.dram_tensor("out", (B, C_OUT, BAND, IMG), f32, kind="ExternalOutput")

    out_r = out_t.ap().rearrange(
        "b c (t q r) w -> b q c t (r w)", t=SUPER, q=2, r=SROWS // 2
    )

    with tile.TileContext(nc) as tc:
        with (
            tc.tile_pool(name="const", bufs=1) as cpool,
            tc.tile_pool(name="imcol", bufs=3) as ipool,
            tc.tile_pool(name="psum", bufs=2, space="PSUM") as ppool,
            tc.tile_pool(name="warm", bufs=1, space="PSUM") as wpool,
            tc.tile_pool(name="tmp", bufs=3) as tpool,
            tc.tile_pool(name="outp", bufs=6) as opool,
        ):
            # HAM warm-up: ~7us of dense back-to-back matmuls on dummy
            # data while the first imcol DMA is in flight. Without this
            # the PE clock-gate (K=4/8, 1.2 GHz) takes ~90us to release
            # (or never does) and every matmul runs 2-4x slow.
            dummy = cpool.tile([128, NBLK], bf16)
            nc.vector.memset(dummy[:], 0)
            warm_ps = wpool.tile([128, NBLK], f32, tag="warm")
            for _ in range(24):
                nc.tensor.matmul(
                    warm_ps[:], dummy[:, 0:128], dummy[:],
                    start=True, stop=True, skip_group_check=True,
                )
            # order matters: imcol(0) gates the first matmul -> sync ring,
            # first. consts on the ACT ring. svb halves late (first needed
            # by blend(0), ~3us after the first matmul).
            # first chunk covers supertiles 0-1 (rows 0..7) so the first
            # real matmul isn't gated on the whole batch
            imcols = {}
            imc = ipool.tile([96, IMW], bf16, tag="imcol")
            cut = 8 * WP
            nc.sync.dma_start(imc[:, 0:cut], xs_t.ap()[0][:, 0:cut])
            nc.sync.dma_start(imc[:, cut:], xs_t.ap()[0][:, cut:])
            imcols[0] = imc

            wtbc_sb = cpool.tile([96, 3, 128], bf16)
            nc.scalar.dma_start(
                wtbc_sb[:],
                wpack_t.ap()[0:96, _OFF_WTBC : _OFF_WTBC + 384].rearrange(
                    "p (k m) -> p k m", k=3
                ),
            )
            wta_sb = cpool.tile([96, 3, 64], bf16)
            nc.scalar.dma_start(
                wta_sb[:],
                wpack_t.ap()[0:96, _OFF_WTA : _OFF_WTA + 192].rearrange(
                    "p (k m) -> p k m", k=3
                ),
            )
            idt_sb = cpool.tile([128, 64], bf16)
            nc.scalar.dma_start(idt_sb[:], wpack_t.ap()[:, _OFF_ID : _OFF_ID + 64])

            # svb behind imcol(0) on the sync ring: imcol gates the first
            # real matmul; svb is only needed by blend(0), a bit later.
            svb_sb = cpool.tile([128, SUPER, 2 * NBLK], bf16)
            svb_r = svb_t.ap().rearrange("p (t j) -> p t j", t=SUPER)
            for h in range(2):
                nc.sync.dma_start(
                    svb_sb[:, 4 * h : 4 * h + 4, :], svb_r[:, 4 * h : 4 * h + 4, :]
                )

            def wtbc(kw):
                return wtbc_sb[:, kw, :]

            def wta(kw):
                return wta_sb[:, kw, :]

            id_sb = idt_sb[:]

            # Flat software pipeline over (b, t): conv stage at i, blend
            # stage at i-1 so the PE never waits on DVE.
            steps = [(b, t) for b in range(B) for t in range(SUPER)]
            conv_state = {}  # i -> (bc, a2)

            for i, (b, t) in enumerate(steps):
                if t == 2 and b + 1 < B:
                    # prefetch next batch's imcol mid-batch (keeps the
                    # head clear; needed ~5 supertiles later)
                    imc = ipool.tile([96, IMW], bf16, tag="imcol")
                    nc.sync.dma_start(imc[:], xs_t.ap()[b + 1])
                    imcols[b + 1] = imc

                imv = imcols[b].rearrange("p (h w) -> p h w", w=WP)

                # ---- conv stage for step i ----
                bc = ppool.tile([128, 2, NBLK], f32, tag="bc")
                a2 = ppool.tile([128, NBLK], f32, tag="a2")
                hl = SROWS * t
                for kw in range(3):
                    rhs0 = imv[:, hl : hl + 2, kw : kw + IMG]
                    rhs1 = imv[:, hl + 2 : hl + 4, kw : kw + IMG]
                    nc.tensor.matmul(
                        bc[:, 0, :], wtbc(kw), rhs0,
                        start=(kw == 0), stop=(kw == 2),
                    )
                    nc.tensor.matmul(
                        bc[:, 1, :], wtbc(kw), rhs1,
                        start=(kw == 0), stop=(kw == 2),
                    )
                    # rank-0 for both blocks: col-tiled concurrent pair
                    nc.tensor.matmul(
                        a2[0:64, :], wta(kw), rhs0,
                        start=(kw == 0), stop=False, skip_group_check=True,
                    )
                    nc.tensor.matmul(
                        a2[64:128, :], wta(kw), rhs1,
                        start=(kw == 0), stop=False, skip_group_check=True,
                    )
                conv_state[i] = (bc, a2)

                # ---- blend stage for step i-1 ----
                if i >= 1:
                    _blend(nc, tpool, opool, conv_state, i - 1, steps,
                           svb_sb, id_sb, out_r, f32)
                    del conv_state[i - 1]

            _blend(nc, tpool, opool, conv_state, len(steps) - 1, steps,
                   svb_sb, id_sb, out_r, f32, split=True)

    nc.compile()
    return nc


def _blend(nc, tpool, opool, conv_state, i, steps, svb_sb, id_sb, out_r, f32,
           split=False):
    import concourse.mybir as mybir

    bf16 = mybir.dt.bfloat16
    b, t = steps[i]
    bc, a2 = conv_state[i]
    m = tpool.tile([128, 2, NBLK], bf16, tag="m")
    svq = svb_sb[:, t, :].rearrange("p (q j) -> p q j", q=2)
    if split:
        # tail latency: fold q0 while q1's multiply still runs
        for q in range(2):
            nc.vector.tensor_tensor(
                m[:, q, :], bc[:, q, :], svq[:, q, :], mybir.AluOpType.mult
            )
            nc.tensor.matmul(
                a2[64 * q : 64 * q + 64, :], id_sb, m[:, q, :],
                start=False, stop=(q == 1), skip_group_check=True,
            )
    else:
        nc.vector.tensor_tensor(m[:], bc[:], svq, mybir.AluOpType.mult)
        # fold m's halves into a2: col-tiled concurrent pair
        nc.tensor.matmul(
            a2[0:64, :], id_sb, m[:, 0, :],
            start=False, stop=False, skip_group_check=True,
        )
        nc.tensor.matmul(
            a2[64:128, :], id_sb, m[:, 1, :],
            start=False, stop=True, skip_group_check=True,
        )
    out_sb = opool.tile([128, NBLK], f32, tag="out_sb")
    nc.scalar.copy(out_sb[:], a2[:])
    for q in range(2):
        nc.sync.dma_start(out_r[b, q, :, t, :], out_sb[64 * q : 64 * q + 64, :])


_CACHE = {}


def _get_bass():
    if "nc" not in _CACHE:
        _CACHE["nc"] = _build_bass()
    return _CACHE["nc"]


def _prep_shards(x, conv_w, kernel_weight):
    x = np.asarray(x, dtype=_F32)
    conv_w = np.asarray(conv_w, dtype=_F32)
    kernel_weight = np.asarray(kernel_weight, dtype=_F32)

    x_pad = np.pad(x, ((0, 0), (0, 0), (1, 1), (1, 1)))
    # w[kh, c, kw, (r, m)] from conv_w[(r m), c, kh, kw]
    wt = conv_w.transpose(2, 1, 3, 0).reshape(96, 3, RANK * C_OUT)
    wpack = np.zeros((128, _WPACK_W), dtype=_F32)
    for kw in range(3):
        wpack[0:96, _OFF_WTBC + 128 * kw : _OFF_WTBC + 128 * (kw + 1)] = (
            wt[:, kw, C_OUT:]
        )
        wpack[0:96, _OFF_WTA + 64 * kw : _OFF_WTA + 64 * (kw + 1)] = wt[:, kw, :C_OUT]
    eye = np.eye(64, dtype=_F32)
    wpack[0:64, _OFF_ID : _OFF_ID + 64] = eye
    wpack[64:128, _OFF_ID : _OFF_ID + 64] = eye
    wpack = wpack.astype(_BF16)

    in_maps = []
    for i in range(N_CORES):
        h0 = BAND * i
        band_x = x_pad[:, :, h0 : h0 + ROWS_IN, :]          # [B, 32, 34, 258]
        # imcol source: partition (kh, c) = rows kh..kh+BAND-1 flattened
        shard = np.empty((B, 96, IMW), dtype=_F32)
        for kh in range(3):
            shard[:, 32 * kh : 32 * kh + 32, :] = band_x[
                :, :, kh : kh + BAND, :
            ].reshape(B, 32, IMW)
        shard = shard.astype(_BF16)

        band = kernel_weight[:, h0 : h0 + BAND, :]          # [2, 32, 256]
        # svb[64r+c, (t, q, j)] = band[r, row(t, q, j)]
        arr = band.reshape(2, SUPER, 2 * NBLK)              # [r, t, (q j)]
        svb = np.broadcast_to(
            arr[:, None, :, :], (2, C_OUT, SUPER, 2 * NBLK)
        ).reshape(128, SUPER * 2 * NBLK)
        svb = np.ascontiguousarray(svb).astype(_BF16)
        in_maps.append({"xs": shard, "wpack": wpack, "svb": svb})
    return in_maps


def run(inputs, trace=False):
    """Run the sharded bass kernel; returns (out_full, BassKernelResults)."""
    from concourse.bass_utils import run_bass_kernel_spmd

    in_maps = _prep_shards(**inputs)
    nc = _get_bass()
    res = run_bass_kernel_spmd(
        nc, in_maps, core_ids=list(range(N_CORES)), trace=trace
    )
    out = np.empty((B, C_OUT, IMG, IMG), dtype=_F32)
    for i in range(N_CORES):
        out[:, :, BAND * i : BAND * (i + 1), :] = res.results[i]["out"]
    return out, res


def kernel(x, conv_w, kernel_weight):
    out, _ = run({"x": x, "conv_w": conv_w, "kernel_weight": kernel_weight})
    return out


# revision 25
# speedup vs baseline: 1.4023x; 1.4023x over previous
"""Trainium2 Bass kernel for nn_LRSVConv (low-rank spatially-varying conv).

Computes, for full inputs
    x            [8, 32, 256, 256]  f32
    conv_w       [192, 32, 3, 3]    f32   (192 = RANK(3) * C_OUT(64))
    kernel_weight[2, 256, 256]      f32
the reference:
    y   = conv2d(x, conv_w, stride 1, pad 1)      # [8, 192, 256, 256]
    y   = y.reshape(8, 3, 64, 256, 256)
    out = y[:,0] + kw[0]*y[:,1] + kw[1]*y[:,2]    # [8, 64, 256, 256]

Sharding: spatial (H) bands of 32 output rows across the 8 cores, all
batches per core, so the per-pixel blend weights are loaded once per core.

Per core, per supertile (4 output rows = 2 blocks q of 512 px):
  - conv matmuls in bf16 (FWL-able weight loads; rel err 3.4e-3 vs the
    2e-2 gate), K=96 (3 kh x 32 c_in), kw via free-dim shifts of the
    imcol tile (kh replicas pre-built host-side so each batch is ONE
    dma_start):
      per kw: bc[q0] (M=128: ranks 1|2), bc[q1], and a col-tiled
      concurrent pair a2[0:64] / a2[64:128] (M=64: rank 0, blocks q0/q1)
  - blend: m = bc * svb on DVE, then a col-tiled pair of identity
    matmuls folds m's two 64-row halves into a2 (which holds rank-0)
    -> out = y0 + sv1*y1 + sv2*y2 in PSUM; ACT evacuates to SBUF.
  - blend stage runs one supertile behind the conv stage so the PE
    never waits for DVE.
  - DMA: per-batch imcol prefetched mid-previous-batch; consts merged
    into single dma_starts (the HWDGE sequencer costs ~700ns per
    dma_start, and loads FIFO behind each other).
"""

import numpy as np
import ml_dtypes

B, C_IN, C_OUT, RANK, IMG = 8, 32, 64, 3, 256
N_CORES = 8
BAND = IMG // N_CORES          # 32 output rows per core
WP = IMG + 2                   # padded width 258
ROWS_IN = BAND + 2             # input rows needed per band (with halo)
SUPER = 8                      # supertiles per (batch, band): 4 rows each
SROWS = BAND // SUPER          # 4 image rows per supertile
NBLK = 512                     # pixels per matmul block (2 image rows)
IMW = BAND * WP                # imcol free size per kh block (8256)

_F32 = np.float32
_BF16 = ml_dtypes.bfloat16

# packed const layout (free-dim offsets in the [128, 640] wpack tile)
_OFF_WTBC = 0            # [0:96, kw*128 : kw*128+128], kw = 0..2
_OFF_WTA = 384           # [0:96, 384+kw*64 : +64]
_OFF_ID = 576            # [0:128, 576:640] = [I64; I64]
_WPACK_W = 640


def _build_bass():
    import concourse.mybir as mybir
    import concourse.tile as tile
    from concourse import bacc

    f32 = mybir.dt.float32
    bf16 = mybir.dt.bfloat16
    nc = bacc.Bacc("TRN2", target_bir_lowering=False, debug=False)

    # xs: host-prepped imcol source, partition dim (kh, c) pre-replicated
    xs_t = nc.dram_tensor("xs", (B, 96, IMW), bf16, kind="ExternalInput")
    wpack_t = nc.dram_tensor("wpack", (128, _WPACK_W), bf16, kind="ExternalInput")
    # svb rows 0:64 = sv1 (bcast over c), rows 64:128 = sv2; cols = (t, q, j)
    svb_t = nc.dram_tensor("svb", (128, SUPER * 2 * NBLK), bf16, kind="ExternalInput")
    out_t = nc.dram_tensor("out", (B, C_OUT, BAND, IMG), f32, kind="ExternalOutput")

    out_r = out_t.ap().rearrange(
        "b c (t q r) w -> b q c t (r w)", t=SUPER, q=2, r=SROWS // 2
    )

    with tile.TileContext(nc) as tc:
        with (
            tc.tile_pool(name="const", bufs=1) as cpool,
            tc.tile_pool(name="imcol", bufs=3) as ipool,
            tc.tile_pool(name="psum", bufs=2, space="PSUM") as ppool,
            tc.tile_pool(name="warm", bufs=1, space="PSUM") as wpool,
            tc.tile_pool(name="tmp", bufs=3) as tpool,
            tc.tile_pool(name="outp", bufs=6) as opool,
        ):
            # HAM warm-up: ~7us of dense back-to-back matmuls on dummy
            # data while the first imcol DMA is in flight. Without this
            # the PE clock-gate (K=4/8, 1.2 GHz) takes ~90us to release
            # (or never does) and every matmul runs 2-4x slow.
            dummy = cpool.tile([128, NBLK], bf16)
            nc.vector.memset(dummy[:], 0)
            warm_ps = wpool.tile([128, NBLK], f32, tag="warm")
            for _ in range(48):
                nc.tensor.matmul(
                    warm_ps[:], dummy[:, 0:128], dummy[:],
                    start=True, stop=True, skip_group_check=True,
                )
            # order matters: imcol(0) gates the first matmul -> sync ring,
            # first. consts on the ACT ring. svb halves late (first needed
            # by blend(0), ~3us after the first matmul).
            # first chunk covers supertiles 0-1 (rows 0..7) so the first
            # real matmul isn't gated on the whole batch
            imcols = {}
            imc = ipool.tile([96, IMW], bf16, tag="imcol")
            cut = 8 * WP
            nc.sync.dma_start(imc[:, 0:cut], xs_t.ap()[0][:, 0:cut])
            imcols[0] = imc

            wtbc_sb = cpool.tile([96, 3, 128], bf16)
            nc.scalar.dma_start(
                wtbc_sb[:],
                wpack_t.ap()[0:96, _OFF_WTBC : _OFF_WTBC + 384].rearrange(
                    "p (k m) -> p k m", k=3
                ),
            )
            wta_sb = cpool.tile([96, 3, 64], bf16)
            nc.scalar.dma_start(
                wta_sb[:],
                wpack_t.ap()[0:96, _OFF_WTA : _OFF_WTA + 192].rearrange(
                    "p (k m) -> p k m", k=3
                ),
            )
            idt_sb = cpool.tile([128, 64], bf16)
            nc.scalar.dma_start(idt_sb[:], wpack_t.ap()[:, _OFF_ID : _OFF_ID + 64])

            # sync-ring order = deadline order: imcol rows 0-7 (supertile
            # 0-1), svb half 1 (blend 0-3), imcol rest (supertile 2+),
            # svb half 2. Any PE bubble >~3us re-throttles the clock gate
            # permanently, so every load must land with margin.
            svb_sb = cpool.tile([128, SUPER, 2 * NBLK], bf16)
            svb_r = svb_t.ap().rearrange("p (t j) -> p t j", t=SUPER)
            nc.sync.dma_start(svb_sb[:, 0:4, :], svb_r[:, 0:4, :])
            nc.sync.dma_start(imc[:, cut:], xs_t.ap()[0][:, cut:])
            nc.sync.dma_start(svb_sb[:, 4:8, :], svb_r[:, 4:8, :])

            def wtbc(kw):
                return wtbc_sb[:, kw, :]

            def wta(kw):
                return wta_sb[:, kw, :]

            id_sb = idt_sb[:]

            # Flat software pipeline over (b, t): conv stage at i, blend
            # stage at i-1 so the PE never waits on DVE.
            steps = [(b, t) for b in range(B) for t in range(SUPER)]
            conv_state = {}  # i -> (bc, a2)

            for i, (b, t) in enumerate(steps):
                if t == 2 and b + 1 < B:
                    # prefetch next batch's imcol mid-batch (keeps the
                    # head clear; needed ~5 supertiles later)
                    imc = ipool.tile([96, IMW], bf16, tag="imcol")
                    nc.sync.dma_start(imc[:], xs_t.ap()[b + 1])
                    imcols[b + 1] = imc

                imv = imcols[b].rearrange("p (h w) -> p h w", w=WP)

                # ---- conv stage for step i ----
                bc = ppool.tile([128, 2, NBLK], f32, tag="bc")
                a2 = ppool.tile([128, NBLK], f32, tag="a2")
                hl = SROWS * t
                for kw in range(3):
                    rhs0 = imv[:, hl : hl + 2, kw : kw + IMG]
                    rhs1 = imv[:, hl + 2 : hl + 4, kw : kw + IMG]
                    nc.tensor.matmul(
                        bc[:, 0, :], wtbc(kw), rhs0,
                        start=(kw == 0), stop=(kw == 2),
                    )
                    nc.tensor.matmul(
                        bc[:, 1, :], wtbc(kw), rhs1,
                        start=(kw == 0), stop=(kw == 2),
                    )
                    # rank-0 for both blocks: col-tiled concurrent pair
                    nc.tensor.matmul(
                        a2[0:64, :], wta(kw), rhs0,
                        start=(kw == 0), stop=False, skip_group_check=True,
                    )
                    nc.tensor.matmul(
                        a2[64:128, :], wta(kw), rhs1,
                        start=(kw == 0), stop=False, skip_group_check=True,
                    )
                conv_state[i] = (bc, a2)

                # ---- blend stage for step i-1 ----
                if i >= 1:
                    _blend(nc, tpool, opool, conv_state, i - 1, steps,
                           svb_sb, id_sb, out_r, f32)
                    del conv_state[i - 1]

            _blend(nc, tpool, opool, conv_state, len(steps) - 1, steps,
                   svb_sb, id_sb, out_r, f32, split=True)

    nc.compile()
    return nc


def _blend(nc, tpool, opool, conv_state, i, steps, svb_sb, id_sb, out_r, f32,
           split=False):
    import concourse.mybir as mybir

    bf16 = mybir.dt.bfloat16
    b, t = steps[i]
    bc, a2 = conv_state[i]
    m = tpool.tile([128, 2, NBLK], bf16, tag="m")
    svq = svb_sb[:, t, :].rearrange("p (q j) -> p q j", q=2)
    if split:
        # tail latency: fold q0 while q1's multiply still runs
        for q in range(2):
            nc.vector.tensor_tensor(
                m[:, q, :], bc[:, q, :], svq[:, q, :], mybir.AluOpType.mult
            )
            nc.tensor.matmul(
                a2[64 * q : 64 * q + 64, :], id_sb, m[:, q, :],
                start=False, stop=(q == 1), skip_group_check=True,
            )
    else:
        nc.vector.tensor_tensor(m[:], bc[:], svq, mybir.AluOpType.mult)
        # fold m's halves into a2: col-tiled concurrent pair
        nc.tensor.matmul(
            a2[0:64, :], id_sb, m[:, 0, :],
            start=False, stop=False, skip_group_check=True,
        )
        nc.tensor.matmul(
            a2[64:128, :], id_sb, m[:, 1, :],
            start=False, stop=True, skip_group_check=True,
        )
    out_sb = opool.tile([128, NBLK], f32, tag="out_sb")
    nc.scalar.copy(out_sb[:], a2[:])
    for q in range(2):
        nc.sync.dma_start(out_r[b, q, :, t, :], out_sb[64 * q : 64 * q + 64, :])


_CACHE = {}


def _get_bass():
    if "nc" not in _CACHE:
        _CACHE["nc"] = _build_bass()
    return _CACHE["nc"]


def _prep_shards(x, conv_w, kernel_weight):
    x = np.asarray(x, dtype=_F32)
    conv_w = np.asarray(conv_w, dtype=_F32)
    kernel_weight = np.asarray(kernel_weight, dtype=_F32)

    x_pad = np.pad(x, ((0, 0), (0, 0), (1, 1), (1, 1)))
    # w[kh, c, kw, (r, m)] from conv_w[(r m), c, kh, kw]
    wt = conv_w.transpose(2, 1, 3, 0).reshape(96, 3, RANK * C_OUT)
    wpack = np.zeros((128, _WPACK_W), dtype=_F32)
    for kw in range(3):
        wpack[0:96, _OFF_WTBC + 128 * kw : _OFF_WTBC + 128 * (kw + 1)] = (
            wt[:, kw, C_OUT:]
        )
        wpack[0:96, _OFF_WTA + 64 * kw : _OFF_WTA + 64 * (kw + 1)] = wt[:, kw, :C_OUT]
    eye = np.eye(64, dtype=_F32)
    wpack[0:64, _OFF_ID : _OFF_ID + 64] = eye
    wpack[64:128, _OFF_ID : _OFF_ID + 64] = eye
    wpack = wpack.astype(_BF16)

    in_maps = []
    for i in range(N_CORES):
        h0 = BAND * i
        band_x = x_pad[:, :, h0 : h0 + ROWS_IN, :]          # [B, 32, 34, 258]
        # imcol source: partition (kh, c) = rows kh..kh+BAND-1 flattened
        shard = np.empty((B, 96, IMW), dtype=_F32)
        for kh in range(3):
            shard[:, 32 * kh : 32 * kh + 32, :] = band_x[
                :, :, kh : kh + BAND, :
            ].reshape(B, 32, IMW)
        shard = shard.astype(_BF16)

        band = kernel_weight[:, h0 : h0 + BAND, :]          # [2, 32, 256]
        # svb[64r+c, (t, q, j)] = band[r, row(t, q, j)]
        arr = band.reshape(2, SUPER, 2 * NBLK)              # [r, t, (q j)]
        svb = np.broadcast_to(
            arr[:, None, :, :], (2, C_OUT, SUPER, 2 * NBLK)
        ).reshape(128, SUPER * 2 * NBLK)
        svb = np.ascontiguousarray(svb).astype(_BF16)
        in_maps.append({"xs": shard, "wpack": wpack, "svb": svb})
    return in_maps


def run(inputs, trace=False):
    """Run the sharded bass kernel; returns (out_full, BassKernelResults)."""
    from concourse.bass_utils import run_bass_kernel_spmd

    in_maps = _prep_shards(**inputs)
    nc = _get_bass()
    res = run_bass_kernel_spmd(
        nc, in_maps, core_ids=list(range(N_CORES)), trace=trace
    )
    out = np.empty((B, C_OUT, IMG, IMG), dtype=_F32)
    for i in range(N_CORES):
        out[:, :, BAND * i : BAND * (i + 1), :] = res.results[i]["out"]
    return out, res


def kernel(x, conv_w, kernel_weight):
    out, _ = run({"x": x, "conv_w": conv_w, "kernel_weight": kernel_weight})
    return out


# revision 26
# speedup vs baseline: 1.6945x; 1.2084x over previous
"""Trainium2 Bass kernel for nn_LRSVConv (low-rank spatially-varying conv).

Computes, for full inputs
    x            [8, 32, 256, 256]  f32
    conv_w       [192, 32, 3, 3]    f32   (192 = RANK(3) * C_OUT(64))
    kernel_weight[2, 256, 256]      f32
the reference:
    y   = conv2d(x, conv_w, stride 1, pad 1)      # [8, 192, 256, 256]
    y   = y.reshape(8, 3, 64, 256, 256)
    out = y[:,0] + kw[0]*y[:,1] + kw[1]*y[:,2]    # [8, 64, 256, 256]

Sharding: spatial (H) bands of 32 output rows across the 8 cores, all
batches per core, so the per-pixel blend weights are loaded once per core.

Per core, per supertile (4 output rows = 2 blocks q of 512 px):
  - conv matmuls in bf16 (FWL-able weight loads; rel err 3.4e-3 vs the
    2e-2 gate), K=96 (3 kh x 32 c_in), kw via free-dim shifts of the
    imcol tile (kh replicas pre-built host-side so each batch is ONE
    dma_start):
      per kw: bc[q0] (M=128: ranks 1|2), bc[q1], and a col-tiled
      concurrent pair a2[0:64] / a2[64:128] (M=64: rank 0, blocks q0/q1)
  - blend: m = bc * svb on DVE, then a col-tiled pair of identity
    matmuls folds m's two 64-row halves into a2 (which holds rank-0)
    -> out = y0 + sv1*y1 + sv2*y2 in PSUM; ACT evacuates to SBUF.
  - blend stage runs one supertile behind the conv stage so the PE
    never waits for DVE.
  - DMA: per-batch imcol prefetched mid-previous-batch; consts merged
    into single dma_starts (the HWDGE sequencer costs ~700ns per
    dma_start, and loads FIFO behind each other).
"""

import numpy as np
import ml_dtypes

B, C_IN, C_OUT, RANK, IMG = 8, 32, 64, 3, 256
N_CORES = 8
BAND = IMG // N_CORES          # 32 output rows per core
WP = IMG + 2                   # padded width 258
ROWS_IN = BAND + 2             # input rows needed per band (with halo)
SUPER = 8                      # supertiles per (batch, band): 4 rows each
SROWS = BAND // SUPER          # 4 image rows per supertile
NBLK = 512                     # pixels per matmul block (2 image rows)
IMW = BAND * WP                # imcol free size per kh block (8256)

_F32 = np.float32
_BF16 = ml_dtypes.bfloat16

# packed const layout (free-dim offsets in the [128, 640] wpack tile)
_OFF_WTBC = 0            # [0:96, kw*128 : kw*128+128], kw = 0..2
_OFF_WTA = 384           # [0:96, 384+kw*64 : +64]
_OFF_ID = 576            # [0:128, 576:640] = [I64; I64]
_WPACK_W = 640


def _build_bass():
    import concourse.mybir as mybir
    import concourse.tile as tile
    from concourse import bacc

    f32 = mybir.dt.float32
    bf16 = mybir.dt.bfloat16
    nc = bacc.Bacc("TRN2", target_bir_lowering=False, debug=False)

    # xs: host-prepped imcol source, partition dim (kh, c) pre-replicated
    xs_t = nc.dram_tensor("xs", (B, 96, IMW), bf16, kind="ExternalInput")
    wpack_t = nc.dram_tensor("wpack", (128, _WPACK_W), bf16, kind="ExternalInput")
    # svb rows 0:64 = sv1 (bcast over c), rows 64:128 = sv2; cols = (t, q, j)
    svb_t = nc.dram_tensor("svb", (128, SUPER * 2 * NBLK), bf16, kind="ExternalInput")
    out_t = nc.dram_tensor("out", (B, C_OUT, BAND, IMG), f32, kind="ExternalOutput")

    out_r = out_t.ap().rearrange(
        "b c (t q r) w -> b q c t (r w)", t=SUPER, q=2, r=SROWS // 2
    )

    with tile.TileContext(nc) as tc:
        with (
            tc.tile_pool(name="const", bufs=1) as cpool,
            tc.tile_pool(name="imcol", bufs=3) as ipool,
            tc.tile_pool(name="psum", bufs=2, space="PSUM") as ppool,
            tc.tile_pool(name="warm", bufs=1, space="PSUM") as wpool,
            tc.tile_pool(name="tmp", bufs=3) as tpool,
            tc.tile_pool(name="outp", bufs=6) as opool,
        ):
            # HAM warm-up: ~7us of dense back-to-back matmuls on dummy
            # data while the first imcol DMA is in flight. Without this
            # the PE clock-gate (K=4/8, 1.2 GHz) takes ~90us to release
            # (or never does) and every matmul runs 2-4x slow.
            dummy = cpool.tile([128, NBLK], bf16)
            nc.vector.memset(dummy[:], 0)
            warm_ps = wpool.tile([128, NBLK], f32, tag="warm")
            for _ in range(48):
                nc.tensor.matmul(
                    warm_ps[:], dummy[:, 0:128], dummy[:],
                    start=True, stop=True, skip_group_check=True,
                )
            # order matters: imcol(0) gates the first matmul -> sync ring,
            # first. consts on the ACT ring. svb halves late (first needed
            # by blend(0), ~3us after the first matmul).
            # first chunk covers supertiles 0-1 (rows 0..7) so the first
            # real matmul isn't gated on the whole batch
            imcols = {}
            imc = ipool.tile([96, IMW], bf16, tag="imcol")
            cut = 8 * WP
            nc.sync.dma_start(imc[:, 0:cut], xs_t.ap()[0][:, 0:cut])
            imcols[0] = imc

            wtbc_sb = cpool.tile([96, 3, 128], bf16)
            nc.scalar.dma_start(
                wtbc_sb[:],
                wpack_t.ap()[0:96, _OFF_WTBC : _OFF_WTBC + 384].rearrange(
                    "p (k m) -> p k m", k=3
                ),
            )
            wta_sb = cpool.tile([96, 3, 64], bf16)
            nc.scalar.dma_start(
                wta_sb[:],
                wpack_t.ap()[0:96, _OFF_WTA : _OFF_WTA + 192].rearrange(
                    "p (k m) -> p k m", k=3
                ),
            )
            idt_sb = cpool.tile([128, 64], bf16)
            nc.scalar.dma_start(idt_sb[:], wpack_t.ap()[:, _OFF_ID : _OFF_ID + 64])

            # sync-ring order = deadline order: imcol rows 0-7 (supertile
            # 0-1), svb half 1 (blend 0-3), imcol rest (supertile 2+),
            # svb half 2. Any PE bubble >~3us re-throttles the clock gate
            # permanently, so every load must land with margin.
            svb_sb = cpool.tile([128, SUPER, 2 * NBLK], bf16)
            svb_r = svb_t.ap().rearrange("p (t j) -> p t j", t=SUPER)
            nc.sync.dma_start(svb_sb[:, 0:4, :], svb_r[:, 0:4, :])
            nc.sync.dma_start(imc[:, cut:], xs_t.ap()[0][:, cut:])
            nc.sync.dma_start(svb_sb[:, 4:8, :], svb_r[:, 4:8, :])

            def wtbc(kw):
                return wtbc_sb[:, kw, :]

            def wta(kw):
                return wta_sb[:, kw, :]

            id_sb = idt_sb[:]

            # Flat software pipeline over (b, t): conv stage at i, blend
            # stage at i-1 so the PE never waits on DVE.
            steps = [(b, t) for b in range(B) for t in range(SUPER)]
            conv_state = {}  # i -> (bc, a2)

            for i, (b, t) in enumerate(steps):
                if t == 2 and b + 1 < B:
                    # prefetch next batch's imcol mid-batch (keeps the
                    # head clear; needed ~5 supertiles later)
                    imc = ipool.tile([96, IMW], bf16, tag="imcol")
                    nc.sync.dma_start(imc[:], xs_t.ap()[b + 1])
                    imcols[b + 1] = imc

                imv = imcols[b].rearrange("p (h w) -> p h w", w=WP)

                # ---- conv stage for step i ----
                bc = ppool.tile([128, 2, NBLK], f32, tag="bc")
                a2 = ppool.tile([128, NBLK], f32, tag="a2")
                hl = SROWS * t
                for kw in range(3):
                    rhs0 = imv[:, hl : hl + 2, kw : kw + IMG]
                    rhs1 = imv[:, hl + 2 : hl + 4, kw : kw + IMG]
                    nc.tensor.matmul(
                        bc[:, 0, :], wtbc(kw), rhs0,
                        start=(kw == 0), stop=(kw == 2),
                    )
                    nc.tensor.matmul(
                        bc[:, 1, :], wtbc(kw), rhs1,
                        start=(kw == 0), stop=(kw == 2),
                    )
                    # rank-0 for both blocks: col-tiled concurrent pair
                    nc.tensor.matmul(
                        a2[0:64, :], wta(kw), rhs0,
                        start=(kw == 0), stop=False, skip_group_check=True,
                    )
                    nc.tensor.matmul(
                        a2[64:128, :], wta(kw), rhs1,
                        start=(kw == 0), stop=False, skip_group_check=True,
                    )
                conv_state[i] = (bc, a2)

                # ---- blend stage for step i-1 ----
                if i >= 1:
                    _blend(nc, tpool, opool, conv_state, i - 1, steps,
                           svb_sb, id_sb, out_r, f32)
                    del conv_state[i - 1]

            _blend(nc, tpool, opool, conv_state, len(steps) - 1, steps,
                   svb_sb, id_sb, out_r, f32, split=True)

    nc.compile()
    return nc


def _blend(nc, tpool, opool, conv_state, i, steps, svb_sb, id_sb, out_r, f32,
           split=False):
    import concourse.mybir as mybir

    bf16 = mybir.dt.bfloat16
    b, t = steps[i]
    bc, a2 = conv_state[i]
    m = tpool.tile([128, 2, NBLK], bf16, tag="m")
    svq = svb_sb[:, t, :].rearrange("p (q j) -> p q j", q=2)
    if split:
        # tail latency: fold q0 while q1's multiply still runs
        for q in range(2):
            nc.vector.tensor_tensor(
                m[:, q, :], bc[:, q, :], svq[:, q, :], mybir.AluOpType.mult
            )
            nc.tensor.matmul(
                a2[64 * q : 64 * q + 64, :], id_sb, m[:, q, :],
                start=False, stop=(q == 1), skip_group_check=True,
            )
    else:
        nc.vector.tensor_tensor(m[:], bc[:], svq, mybir.AluOpType.mult)
        # fold m's halves into a2: col-tiled concurrent pair
        nc.tensor.matmul(
            a2[0:64, :], id_sb, m[:, 0, :],
            start=False, stop=False, skip_group_check=True,
        )
        nc.tensor.matmul(
            a2[64:128, :], id_sb, m[:, 1, :],
            start=False, stop=True, skip_group_check=True,
        )
    out_sb = opool.tile([128, NBLK], f32, tag="out_sb")
    nc.scalar.copy(out_sb[:], a2[:])
    # one store per HWDGE ring: halves per-ring DIRECT2D sequencer load
    nc.sync.dma_start(out_r[b, 0, :, t, :], out_sb[0:64, :])
    nc.scalar.dma_start(out_r[b, 1, :, t, :], out_sb[64:128, :])


_CACHE = {}


def _get_bass():
    if "nc" not in _CACHE:
        _CACHE["nc"] = _build_bass()
    return _CACHE["nc"]


def _prep_shards(x, conv_w, kernel_weight):
    x = np.asarray(x, dtype=_F32)
    conv_w = np.asarray(conv_w, dtype=_F32)
    kernel_weight = np.asarray(kernel_weight, dtype=_F32)

    x_pad = np.pad(x, ((0, 0), (0, 0), (1, 1), (1, 1)))
    # w[kh, c, kw, (r, m)] from conv_w[(r m), c, kh, kw]
    wt = conv_w.transpose(2, 1, 3, 0).reshape(96, 3, RANK * C_OUT)
    wpack = np.zeros((128, _WPACK_W), dtype=_F32)
    for kw in range(3):
        wpack[0:96, _OFF_WTBC + 128 * kw : _OFF_WTBC + 128 * (kw + 1)] = (
            wt[:, kw, C_OUT:]
        )
        wpack[0:96, _OFF_WTA + 64 * kw : _OFF_WTA + 64 * (kw + 1)] = wt[:, kw, :C_OUT]
    eye = np.eye(64, dtype=_F32)
    wpack[0:64, _OFF_ID : _OFF_ID + 64] = eye
    wpack[64:128, _OFF_ID : _OFF_ID + 64] = eye
    wpack = wpack.astype(_BF16)

    in_maps = []
    for i in range(N_CORES):
        h0 = BAND * i
        band_x = x_pad[:, :, h0 : h0 + ROWS_IN, :]          # [B, 32, 34, 258]
        # imcol source: partition (kh, c) = rows kh..kh+BAND-1 flattened
        shard = np.empty((B, 96, IMW), dtype=_F32)
        for kh in range(3):
            shard[:, 32 * kh : 32 * kh + 32, :] = band_x[
                :, :, kh : kh + BAND, :
            ].reshape(B, 32, IMW)
        shard = shard.astype(_BF16)

        band = kernel_weight[:, h0 : h0 + BAND, :]          # [2, 32, 256]
        # svb[64r+c, (t, q, j)] = band[r, row(t, q, j)]
        arr = band.reshape(2, SUPER, 2 * NBLK)              # [r, t, (q j)]
        svb = np.broadcast_to(
            arr[:, None, :, :], (2, C_OUT, SUPER, 2 * NBLK)
        ).reshape(128, SUPER * 2 * NBLK)
        svb = np.ascontiguousarray(svb).astype(_BF16)
        in_maps.append({"xs": shard, "wpack": wpack, "svb": svb})
    return in_maps


def run(inputs, trace=False):
    """Run the sharded bass kernel; returns (out_full, BassKernelResults)."""
    from concourse.bass_utils import run_bass_kernel_spmd

    in_maps = _prep_shards(**inputs)
    nc = _get_bass()
    res = run_bass_kernel_spmd(
        nc, in_maps, core_ids=list(range(N_CORES)), trace=trace
    )
    out = np.empty((B, C_OUT, IMG, IMG), dtype=_F32)
    for i in range(N_CORES):
        out[:, :, BAND * i : BAND * (i + 1), :] = res.results[i]["out"]
    return out, res


def kernel(x, conv_w, kernel_weight):
    out, _ = run({"x": x, "conv_w": conv_w, "kernel_weight": kernel_weight})
    return out


# revision 27
# speedup vs baseline: 1.8316x; 1.0809x over previous
"""Trainium2 Bass kernel for nn_LRSVConv (low-rank spatially-varying conv).

Computes, for full inputs
    x            [8, 32, 256, 256]  f32
    conv_w       [192, 32, 3, 3]    f32   (192 = RANK(3) * C_OUT(64))
    kernel_weight[2, 256, 256]      f32
the reference:
    y   = conv2d(x, conv_w, stride 1, pad 1)      # [8, 192, 256, 256]
    y   = y.reshape(8, 3, 64, 256, 256)
    out = y[:,0] + kw[0]*y[:,1] + kw[1]*y[:,2]    # [8, 64, 256, 256]

Sharding: spatial (H) bands of 32 output rows across the 8 cores, all
batches per core, so the per-pixel blend weights are loaded once per core.

Per core, per supertile (4 output rows = 2 blocks q of 512 px):
  - conv matmuls in bf16 (FWL-able weight loads; rel err 3.4e-3 vs the
    2e-2 gate), K=96 (3 kh x 32 c_in), kw via free-dim shifts of the
    imcol tile (kh replicas pre-built host-side so each batch is ONE
    dma_start):
      per kw: bc[q0] (M=128: ranks 1|2), bc[q1], and a col-tiled
      concurrent pair a2[0:64] / a2[64:128] (M=64: rank 0, blocks q0/q1)
  - blend: m = bc * svb on DVE, then a col-tiled pair of identity
    matmuls folds m's two 64-row halves into a2 (which holds rank-0)
    -> out = y0 + sv1*y1 + sv2*y2 in PSUM; ACT evacuates to SBUF.
  - blend stage runs one supertile behind the conv stage so the PE
    never waits for DVE.
  - DMA: per-batch imcol prefetched mid-previous-batch; consts merged
    into single dma_starts (the HWDGE sequencer costs ~700ns per
    dma_start, and loads FIFO behind each other).
"""

import numpy as np
import ml_dtypes

B, C_IN, C_OUT, RANK, IMG = 8, 32, 64, 3, 256
N_CORES = 8
BAND = IMG // N_CORES          # 32 output rows per core
WP = IMG + 2                   # padded width 258
ROWS_IN = BAND + 2             # input rows needed per band (with halo)
SUPER = 8                      # supertiles per (batch, band): 4 rows each
SROWS = BAND // SUPER          # 4 image rows per supertile
NBLK = 512                     # pixels per matmul block (2 image rows)
IMW = BAND * WP                # imcol free size per kh block (8256)

_F32 = np.float32
_BF16 = ml_dtypes.bfloat16

# packed const layout (free-dim offsets in the [128, 640] wpack tile)
_OFF_WTBC = 0            # [0:96, kw*128 : kw*128+128], kw = 0..2
_OFF_WTA = 384           # [0:96, 384+kw*64 : +64]
_OFF_ID = 576            # [0:128, 576:640] = [I64; I64]
_WPACK_W = 640


def _build_bass():
    import concourse.mybir as mybir
    import concourse.tile as tile
    from concourse import bacc

    f32 = mybir.dt.float32
    bf16 = mybir.dt.bfloat16
    nc = bacc.Bacc("TRN2", target_bir_lowering=False, debug=False)

    # xs: host-prepped imcol source, partition dim (kh, c) pre-replicated
    xs_t = nc.dram_tensor("xs", (B, 96, IMW), bf16, kind="ExternalInput")
    wpack_t = nc.dram_tensor("wpack", (128, _WPACK_W), bf16, kind="ExternalInput")
    # svb rows 0:64 = sv1 (bcast over c), rows 64:128 = sv2; cols = (t, q, j)
    svb_t = nc.dram_tensor("svb", (128, SUPER * 2 * NBLK), bf16, kind="ExternalInput")
    out_t = nc.dram_tensor("out", (B, C_OUT, BAND, IMG), f32, kind="ExternalOutput")

    out_r = out_t.ap().rearrange(
        "b c (t q r) w -> b q c t (r w)", t=SUPER, q=2, r=SROWS // 2
    )

    with tile.TileContext(nc) as tc:
        with (
            tc.tile_pool(name="const", bufs=1) as cpool,
            tc.tile_pool(name="imcol", bufs=3) as ipool,
            tc.tile_pool(name="psum", bufs=2, space="PSUM") as ppool,
            tc.tile_pool(name="warm", bufs=1, space="PSUM") as wpool,
            tc.tile_pool(name="tmp", bufs=3) as tpool,
            tc.tile_pool(name="outp", bufs=6) as opool,
        ):
            # HAM warm-up: ~7us of dense back-to-back matmuls on dummy
            # data while the first imcol DMA is in flight. Without this
            # the PE clock-gate (K=4/8, 1.2 GHz) takes ~90us to release
            # (or never does) and every matmul runs 2-4x slow.
            dummy = cpool.tile([128, NBLK], bf16)
            nc.vector.memset(dummy[:], 0)
            warm_ps = wpool.tile([128, NBLK], f32, tag="warm")
            for _ in range(48):
                nc.tensor.matmul(
                    warm_ps[:], dummy[:, 0:128], dummy[:],
                    start=True, stop=True, skip_group_check=True,
                )
            # order matters: imcol(0) gates the first matmul -> sync ring,
            # first. consts on the ACT ring. svb halves late (first needed
            # by blend(0), ~3us after the first matmul).
            # first chunk covers supertiles 0-1 (rows 0..7) so the first
            # real matmul isn't gated on the whole batch
            imcols = {}
            imc = ipool.tile([96, IMW], bf16, tag="imcol")
            cut = 8 * WP
            nc.sync.dma_start(imc[:, 0:cut], xs_t.ap()[0][:, 0:cut])
            imcols[0] = imc

            wtbc_sb = cpool.tile([96, 3, 128], bf16)
            nc.scalar.dma_start(
                wtbc_sb[:],
                wpack_t.ap()[0:96, _OFF_WTBC : _OFF_WTBC + 384].rearrange(
                    "p (k m) -> p k m", k=3
                ),
            )
            wta_sb = cpool.tile([96, 3, 64], bf16)
            nc.scalar.dma_start(
                wta_sb[:],
                wpack_t.ap()[0:96, _OFF_WTA : _OFF_WTA + 192].rearrange(
                    "p (k m) -> p k m", k=3
                ),
            )
            idt_sb = cpool.tile([128, 64], bf16)
            nc.scalar.dma_start(idt_sb[:], wpack_t.ap()[:, _OFF_ID : _OFF_ID + 64])

            # sync-ring order = deadline order: imcol rows 0-7 (supertile
            # 0-1), svb half 1 (blend 0-3), imcol rest (supertile 2+),
            # svb half 2. Any PE bubble >~3us re-throttles the clock gate
            # permanently, so every load must land with margin.
            svb_sb = cpool.tile([128, SUPER, 2 * NBLK], bf16)
            svb_r = svb_t.ap().rearrange("p (t j) -> p t j", t=SUPER)
            nc.sync.dma_start(svb_sb[:, 0:4, :], svb_r[:, 0:4, :])
            nc.sync.dma_start(imc[:, cut:], xs_t.ap()[0][:, cut:])
            nc.sync.dma_start(svb_sb[:, 4:8, :], svb_r[:, 4:8, :])

            def wtbc(kw):
                return wtbc_sb[:, kw, :]

            def wta(kw):
                return wta_sb[:, kw, :]

            id_sb = idt_sb[:]

            # Flat software pipeline over (b, t): conv stage at i, blend
            # stage at i-1 so the PE never waits on DVE.
            steps = [(b, t) for b in range(B) for t in range(SUPER)]
            conv_state = {}  # i -> (bc, a2)

            for i, (b, t) in enumerate(steps):
                if t == 2 and b + 1 < B:
                    # prefetch next batch's imcol mid-batch (keeps the
                    # head clear; needed ~5 supertiles later)
                    imc = ipool.tile([96, IMW], bf16, tag="imcol")
                    nc.sync.dma_start(imc[:], xs_t.ap()[b + 1])
                    imcols[b + 1] = imc

                imv = imcols[b].rearrange("p (h w) -> p h w", w=WP)

                # ---- conv stage for step i ----
                bc = ppool.tile([128, 2, NBLK], f32, tag="bc")
                a2 = ppool.tile([128, NBLK], f32, tag="a2")
                hl = SROWS * t
                for kw in range(3):
                    rhs0 = imv[:, hl : hl + 2, kw : kw + IMG]
                    rhs1 = imv[:, hl + 2 : hl + 4, kw : kw + IMG]
                    nc.tensor.matmul(
                        bc[:, 0, :], wtbc(kw), rhs0,
                        start=(kw == 0), stop=(kw == 2),
                    )
                    nc.tensor.matmul(
                        bc[:, 1, :], wtbc(kw), rhs1,
                        start=(kw == 0), stop=(kw == 2),
                    )
                for kw in range(3):
                    rhs0 = imv[:, hl : hl + 2, kw : kw + IMG]
                    rhs1 = imv[:, hl + 2 : hl + 4, kw : kw + IMG]
                    # rank-0 for both blocks: col-tiled concurrent pair
                    nc.tensor.matmul(
                        a2[0:64, :], wta(kw), rhs0,
                        start=(kw == 0), stop=False, skip_group_check=True,
                    )
                    nc.tensor.matmul(
                        a2[64:128, :], wta(kw), rhs1,
                        start=(kw == 0), stop=False, skip_group_check=True,
                    )
                conv_state[i] = (bc, a2)

                # ---- blend stage for step i-1 ----
                if i >= 1:
                    _blend(nc, tpool, opool, conv_state, i - 1, steps,
                           svb_sb, id_sb, out_r, f32)
                    del conv_state[i - 1]

            _blend(nc, tpool, opool, conv_state, len(steps) - 1, steps,
                   svb_sb, id_sb, out_r, f32, split=True)

    nc.compile()
    return nc


def _blend(nc, tpool, opool, conv_state, i, steps, svb_sb, id_sb, out_r, f32,
           split=False):
    import concourse.mybir as mybir

    bf16 = mybir.dt.bfloat16
    b, t = steps[i]
    bc, a2 = conv_state[i]
    m = tpool.tile([128, 2, NBLK], bf16, tag="m")
    svq = svb_sb[:, t, :].rearrange("p (q j) -> p q j", q=2)
    if split:
        # tail latency: fold q0 while q1's multiply still runs
        for q in range(2):
            nc.vector.tensor_tensor(
                m[:, q, :], bc[:, q, :], svq[:, q, :], mybir.AluOpType.mult
            )
            nc.tensor.matmul(
                a2[64 * q : 64 * q + 64, :], id_sb, m[:, q, :],
                start=False, stop=(q == 1), skip_group_check=True,
            )
    else:
        nc.vector.tensor_tensor(m[:], bc[:], svq, mybir.AluOpType.mult)
        # fold m's halves into a2: col-tiled concurrent pair
        nc.tensor.matmul(
            a2[0:64, :], id_sb, m[:, 0, :],
            start=False, stop=False, skip_group_check=True,
        )
        nc.tensor.matmul(
            a2[64:128, :], id_sb, m[:, 1, :],
            start=False, stop=True, skip_group_check=True,
        )
    out_sb = opool.tile([128, NBLK], f32, tag="out_sb")
    nc.scalar.copy(out_sb[:], a2[:])
    # one store per HWDGE ring: halves per-ring DIRECT2D sequencer load
    nc.sync.dma_start(out_r[b, 0, :, t, :], out_sb[0:64, :])
    nc.scalar.dma_start(out_r[b, 1, :, t, :], out_sb[64:128, :])


_CACHE = {}


def _get_bass():
    if "nc" not in _CACHE:
        _CACHE["nc"] = _build_bass()
    return _CACHE["nc"]


def _prep_shards(x, conv_w, kernel_weight):
    x = np.asarray(x, dtype=_F32)
    conv_w = np.asarray(conv_w, dtype=_F32)
    kernel_weight = np.asarray(kernel_weight, dtype=_F32)

    x_pad = np.pad(x, ((0, 0), (0, 0), (1, 1), (1, 1)))
    # w[kh, c, kw, (r, m)] from conv_w[(r m), c, kh, kw]
    wt = conv_w.transpose(2, 1, 3, 0).reshape(96, 3, RANK * C_OUT)
    wpack = np.zeros((128, _WPACK_W), dtype=_F32)
    for kw in range(3):
        wpack[0:96, _OFF_WTBC + 128 * kw : _OFF_WTBC + 128 * (kw + 1)] = (
            wt[:, kw, C_OUT:]
        )
        wpack[0:96, _OFF_WTA + 64 * kw : _OFF_WTA + 64 * (kw + 1)] = wt[:, kw, :C_OUT]
    eye = np.eye(64, dtype=_F32)
    wpack[0:64, _OFF_ID : _OFF_ID + 64] = eye
    wpack[64:128, _OFF_ID : _OFF_ID + 64] = eye
    wpack = wpack.astype(_BF16)

    in_maps = []
    for i in range(N_CORES):
        h0 = BAND * i
        band_x = x_pad[:, :, h0 : h0 + ROWS_IN, :]          # [B, 32, 34, 258]
        # imcol source: partition (kh, c) = rows kh..kh+BAND-1 flattened
        shard = np.empty((B, 96, IMW), dtype=_F32)
        for kh in range(3):
            shard[:, 32 * kh : 32 * kh + 32, :] = band_x[
                :, :, kh : kh + BAND, :
            ].reshape(B, 32, IMW)
        shard = shard.astype(_BF16)

        band = kernel_weight[:, h0 : h0 + BAND, :]          # [2, 32, 256]
        # svb[64r+c, (t, q, j)] = band[r, row(t, q, j)]
        arr = band.reshape(2, SUPER, 2 * NBLK)              # [r, t, (q j)]
        svb = np.broadcast_to(
            arr[:, None, :, :], (2, C_OUT, SUPER, 2 * NBLK)
        ).reshape(128, SUPER * 2 * NBLK)
        svb = np.ascontiguousarray(svb).astype(_BF16)
        in_maps.append({"xs": shard, "wpack": wpack, "svb": svb})
    return in_maps


def run(inputs, trace=False):
    """Run the sharded bass kernel; returns (out_full, BassKernelResults)."""
    from concourse.bass_utils import run_bass_kernel_spmd

    in_maps = _prep_shards(**inputs)
    nc = _get_bass()
    res = run_bass_kernel_spmd(
        nc, in_maps, core_ids=list(range(N_CORES)), trace=trace
    )
    out = np.empty((B, C_OUT, IMG, IMG), dtype=_F32)
    for i in range(N_CORES):
        out[:, :, BAND * i : BAND * (i + 1), :] = res.results[i]["out"]
    return out, res


def kernel(x, conv_w, kernel_weight):
    out, _ = run({"x": x, "conv_w": conv_w, "kernel_weight": kernel_weight})
    return out


# revision 29
# speedup vs baseline: 1.8356x; 1.0022x over previous
"""Trainium2 Bass kernel for nn_LRSVConv (low-rank spatially-varying conv).

Computes, for full inputs
    x            [8, 32, 256, 256]  f32
    conv_w       [192, 32, 3, 3]    f32   (192 = RANK(3) * C_OUT(64))
    kernel_weight[2, 256, 256]      f32
the reference:
    y   = conv2d(x, conv_w, stride 1, pad 1)      # [8, 192, 256, 256]
    y   = y.reshape(8, 3, 64, 256, 256)
    out = y[:,0] + kw[0]*y[:,1] + kw[1]*y[:,2]    # [8, 64, 256, 256]

Sharding: spatial (H) bands of 32 output rows across the 8 cores, all
batches per core, so the per-pixel blend weights are loaded once per core.

Per core, per supertile (4 output rows = 2 blocks q of 512 px):
  - conv matmuls in bf16 (FWL-able weight loads; rel err 3.4e-3 vs the
    2e-2 gate), K=96 (3 kh x 32 c_in), kw via free-dim shifts of the
    imcol tile (kh replicas pre-built host-side so each batch is ONE
    dma_start):
      per kw: bc[q0] (M=128: ranks 1|2), bc[q1], and a col-tiled
      concurrent pair a2[0:64] / a2[64:128] (M=64: rank 0, blocks q0/q1)
  - blend: m = bc * svb on DVE, then a col-tiled pair of identity
    matmuls folds m's two 64-row halves into a2 (which holds rank-0)
    -> out = y0 + sv1*y1 + sv2*y2 in PSUM; ACT evacuates to SBUF.
  - blend stage runs one supertile behind the conv stage so the PE
    never waits for DVE.
  - DMA: per-batch imcol prefetched mid-previous-batch; consts merged
    into single dma_starts (the HWDGE sequencer costs ~700ns per
    dma_start, and loads FIFO behind each other).
"""

import numpy as np
import ml_dtypes

B, C_IN, C_OUT, RANK, IMG = 8, 32, 64, 3, 256
N_CORES = 8
BAND = IMG // N_CORES          # 32 output rows per core
WP = IMG + 2                   # padded width 258
ROWS_IN = BAND + 2             # input rows needed per band (with halo)
SUPER = 8                      # supertiles per (batch, band): 4 rows each
SROWS = BAND // SUPER          # 4 image rows per supertile
NBLK = 512                     # pixels per matmul block (2 image rows)
IMW = BAND * WP                # imcol free size per kh block (8256)

_F32 = np.float32
_BF16 = ml_dtypes.bfloat16

# packed const layout (free-dim offsets in the [128, 640] wpack tile)
_OFF_WTBC = 0            # [0:96, kw*128 : kw*128+128], kw = 0..2
_OFF_WTA = 384           # [0:96, 384+kw*64 : +64]
_OFF_ID = 576            # [0:128, 576:640] = [I64; I64]
_WPACK_W = 640


def _build_bass():
    import concourse.mybir as mybir
    import concourse.tile as tile
    from concourse import bacc

    f32 = mybir.dt.float32
    bf16 = mybir.dt.bfloat16
    nc = bacc.Bacc("TRN2", target_bir_lowering=False, debug=False)

    # xs: host-prepped imcol source, partition dim (kh, c) pre-replicated
    xs_t = nc.dram_tensor("xs", (B, 96, IMW), bf16, kind="ExternalInput")
    wpack_t = nc.dram_tensor("wpack", (128, _WPACK_W), bf16, kind="ExternalInput")
    # svb rows 0:64 = sv1 (bcast over c), rows 64:128 = sv2; cols = (t, q, j)
    svb_t = nc.dram_tensor("svb", (128, SUPER * 2 * NBLK), bf16, kind="ExternalInput")
    out_t = nc.dram_tensor("out", (B, C_OUT, BAND, IMG), f32, kind="ExternalOutput")

    out_r = out_t.ap().rearrange(
        "b c (t q r) w -> b q c t (r w)", t=SUPER, q=2, r=SROWS // 2
    )

    with tile.TileContext(nc) as tc:
        with (
            tc.tile_pool(name="const", bufs=1) as cpool,
            tc.tile_pool(name="imcol", bufs=3) as ipool,
            tc.tile_pool(name="psum", bufs=2, space="PSUM") as ppool,
            tc.tile_pool(name="warm", bufs=1, space="PSUM") as wpool,
            tc.tile_pool(name="tmp", bufs=3) as tpool,
            tc.tile_pool(name="outp", bufs=6) as opool,
        ):
            # HAM warm-up: ~7us of dense back-to-back matmuls on dummy
            # data while the first imcol DMA is in flight. Without this
            # the PE clock-gate (K=4/8, 1.2 GHz) takes ~90us to release
            # (or never does) and every matmul runs 2-4x slow.
            dummy = cpool.tile([128, NBLK], bf16)
            nc.vector.memset(dummy[:], 0)
            warm_ps = wpool.tile([128, NBLK], f32, tag="warm")
            for _ in range(44):
                nc.tensor.matmul(
                    warm_ps[:], dummy[:, 0:128], dummy[:],
                    start=True, stop=True, skip_group_check=True,
                )
            # order matters: imcol(0) gates the first matmul -> sync ring,
            # first. consts on the ACT ring. svb halves late (first needed
            # by blend(0), ~3us after the first matmul).
            # first chunk covers supertiles 0-1 (rows 0..7) so the first
            # real matmul isn't gated on the whole batch
            imcols = {}
            imc = ipool.tile([96, IMW], bf16, tag="imcol")
            cut = 8 * WP
            nc.sync.dma_start(imc[:, 0:cut], xs_t.ap()[0][:, 0:cut])
            imcols[0] = imc

            wtbc_sb = cpool.tile([96, 3, 128], bf16)
            nc.scalar.dma_start(
                wtbc_sb[:],
                wpack_t.ap()[0:96, _OFF_WTBC : _OFF_WTBC + 384].rearrange(
                    "p (k m) -> p k m", k=3
                ),
            )
            wta_sb = cpool.tile([96, 3, 64], bf16)
            nc.scalar.dma_start(
                wta_sb[:],
                wpack_t.ap()[0:96, _OFF_WTA : _OFF_WTA + 192].rearrange(
                    "p (k m) -> p k m", k=3
                ),
            )
            idt_sb = cpool.tile([128, 64], bf16)
            nc.scalar.dma_start(idt_sb[:], wpack_t.ap()[:, _OFF_ID : _OFF_ID + 64])

            # sync-ring order = deadline order: imcol rows 0-7 (supertile
            # 0-1), svb half 1 (blend 0-3), imcol rest (supertile 2+),
            # svb half 2. Any PE bubble >~3us re-throttles the clock gate
            # permanently, so every load must land with margin.
            svb_sb = cpool.tile([128, SUPER, 2 * NBLK], bf16)
            svb_r = svb_t.ap().rearrange("p (t j) -> p t j", t=SUPER)
            nc.sync.dma_start(svb_sb[:, 0:4, :], svb_r[:, 0:4, :])
            nc.sync.dma_start(imc[:, cut:], xs_t.ap()[0][:, cut:])
            nc.sync.dma_start(svb_sb[:, 4:8, :], svb_r[:, 4:8, :])

            def wtbc(kw):
                return wtbc_sb[:, kw, :]

            def wta(kw):
                return wta_sb[:, kw, :]

            id_sb = idt_sb[:]

            # Flat software pipeline over (b, t): conv stage at i, blend
            # stage at i-1 so the PE never waits on DVE.
            steps = [(b, t) for b in range(B) for t in range(SUPER)]
            conv_state = {}  # i -> (bc, a2)

            for i, (b, t) in enumerate(steps):
                if t == 2 and b + 1 < B:
                    # prefetch next batch's imcol mid-batch (keeps the
                    # head clear; needed ~5 supertiles later)
                    imc = ipool.tile([96, IMW], bf16, tag="imcol")
                    nc.sync.dma_start(imc[:], xs_t.ap()[b + 1])
                    imcols[b + 1] = imc

                imv = imcols[b].rearrange("p (h w) -> p h w", w=WP)

                # ---- conv stage for step i ----
                bc = ppool.tile([128, 2, NBLK], f32, tag="bc")
                a2 = ppool.tile([128, NBLK], f32, tag="a2")
                hl = SROWS * t
                for kw in range(3):
                    rhs0 = imv[:, hl : hl + 2, kw : kw + IMG]
                    rhs1 = imv[:, hl + 2 : hl + 4, kw : kw + IMG]
                    nc.tensor.matmul(
                        bc[:, 0, :], wtbc(kw), rhs0,
                        start=(kw == 0), stop=(kw == 2),
                    )
                    nc.tensor.matmul(
                        bc[:, 1, :], wtbc(kw), rhs1,
                        start=(kw == 0), stop=(kw == 2),
                    )
                # blend of the previous step between the bc group and the
                # a2 group: its ident pair lands adjacent to the a2 pairs
                # (pair-block), and m(i-1) has had conv(i)'s bc time to
                # finish on DVE.
                if i >= 1:
                    _blend(nc, tpool, opool, conv_state, i - 1, steps,
                           svb_sb, id_sb, out_r, f32)
                    del conv_state[i - 1]

                for kw in range(3):
                    rhs0 = imv[:, hl : hl + 2, kw : kw + IMG]
                    rhs1 = imv[:, hl + 2 : hl + 4, kw : kw + IMG]
                    # rank-0 for both blocks: col-tiled concurrent pair
                    nc.tensor.matmul(
                        a2[0:64, :], wta(kw), rhs0,
                        start=(kw == 0), stop=False, skip_group_check=True,
                    )
                    nc.tensor.matmul(
                        a2[64:128, :], wta(kw), rhs1,
                        start=(kw == 0), stop=False, skip_group_check=True,
                    )
                conv_state[i] = (bc, a2)

            _blend(nc, tpool, opool, conv_state, len(steps) - 1, steps,
                   svb_sb, id_sb, out_r, f32, split=True)

    nc.compile()
    return nc


def _blend(nc, tpool, opool, conv_state, i, steps, svb_sb, id_sb, out_r, f32,
           split=False):
    import concourse.mybir as mybir

    bf16 = mybir.dt.bfloat16
    b, t = steps[i]
    bc, a2 = conv_state[i]
    m = tpool.tile([128, 2, NBLK], bf16, tag="m")
    svq = svb_sb[:, t, :].rearrange("p (q j) -> p q j", q=2)
    if split:
        # tail latency: fold q0 while q1's multiply still runs
        for q in range(2):
            nc.vector.tensor_tensor(
                m[:, q, :], bc[:, q, :], svq[:, q, :], mybir.AluOpType.mult
            )
            nc.tensor.matmul(
                a2[64 * q : 64 * q + 64, :], id_sb, m[:, q, :],
                start=False, stop=(q == 1), skip_group_check=True,
            )
    else:
        nc.vector.tensor_tensor(m[:], bc[:], svq, mybir.AluOpType.mult)
        # fold m's halves into a2: col-tiled concurrent pair
        nc.tensor.matmul(
            a2[0:64, :], id_sb, m[:, 0, :],
            start=False, stop=False, skip_group_check=True,
        )
        nc.tensor.matmul(
            a2[64:128, :], id_sb, m[:, 1, :],
            start=False, stop=True, skip_group_check=True,
        )
    out_sb = opool.tile([128, NBLK], f32, tag="out_sb")
    nc.scalar.copy(out_sb[:], a2[:])
    # one store per HWDGE ring: halves per-ring DIRECT2D sequencer load
    nc.sync.dma_start(out_r[b, 0, :, t, :], out_sb[0:64, :])
    nc.scalar.dma_start(out_r[b, 1, :, t, :], out_sb[64:128, :])


_CACHE = {}


def _get_bass():
    if "nc" not in _CACHE:
        _CACHE["nc"] = _build_bass()
    return _CACHE["nc"]


def _prep_shards(x, conv_w, kernel_weight):
    x = np.asarray(x, dtype=_F32)
    conv_w = np.asarray(conv_w, dtype=_F32)
    kernel_weight = np.asarray(kernel_weight, dtype=_F32)

    x_pad = np.pad(x, ((0, 0), (0, 0), (1, 1), (1, 1)))
    # w[kh, c, kw, (r, m)] from conv_w[(r m), c, kh, kw]
    wt = conv_w.transpose(2, 1, 3, 0).reshape(96, 3, RANK * C_OUT)
    wpack = np.zeros((128, _WPACK_W), dtype=_F32)
    for kw in range(3):
        wpack[0:96, _OFF_WTBC + 128 * kw : _OFF_WTBC + 128 * (kw + 1)] = (
            wt[:, kw, C_OUT:]
        )
        wpack[0:96, _OFF_WTA + 64 * kw : _OFF_WTA + 64 * (kw + 1)] = wt[:, kw, :C_OUT]
    eye = np.eye(64, dtype=_F32)
    wpack[0:64, _OFF_ID : _OFF_ID + 64] = eye
    wpack[64:128, _OFF_ID : _OFF_ID + 64] = eye
    wpack = wpack.astype(_BF16)

    in_maps = []
    for i in range(N_CORES):
        h0 = BAND * i
        band_x = x_pad[:, :, h0 : h0 + ROWS_IN, :]          # [B, 32, 34, 258]
        # imcol source: partition (kh, c) = rows kh..kh+BAND-1 flattened
        shard = np.empty((B, 96, IMW), dtype=_F32)
        for kh in range(3):
            shard[:, 32 * kh : 32 * kh + 32, :] = band_x[
                :, :, kh : kh + BAND, :
            ].reshape(B, 32, IMW)
        shard = shard.astype(_BF16)

        band = kernel_weight[:, h0 : h0 + BAND, :]          # [2, 32, 256]
        # svb[64r+c, (t, q, j)] = band[r, row(t, q, j)]
        arr = band.reshape(2, SUPER, 2 * NBLK)              # [r, t, (q j)]
        svb = np.broadcast_to(
            arr[:, None, :, :], (2, C_OUT, SUPER, 2 * NBLK)
        ).reshape(128, SUPER * 2 * NBLK)
        svb = np.ascontiguousarray(svb).astype(_BF16)
        in_maps.append({"xs": shard, "wpack": wpack, "svb": svb})
    return in_maps


def run(inputs, trace=False):
    """Run the sharded bass kernel; returns (out_full, BassKernelResults)."""
    from concourse.bass_utils import run_bass_kernel_spmd

    in_maps = _prep_shards(**inputs)
    nc = _get_bass()
    res = run_bass_kernel_spmd(
        nc, in_maps, core_ids=list(range(N_CORES)), trace=trace
    )
    out = np.empty((B, C_OUT, IMG, IMG), dtype=_F32)
    for i in range(N_CORES):
        out[:, :, BAND * i : BAND * (i + 1), :] = res.results[i]["out"]
    return out, res


def kernel(x, conv_w, kernel_weight):
    out, _ = run({"x": x, "conv_w": conv_w, "kernel_weight": kernel_weight})
    return out


# revision 30
# speedup vs baseline: 1.8463x; 1.0058x over previous
"""Trainium2 Bass kernel for nn_LRSVConv (low-rank spatially-varying conv).

Computes, for full inputs
    x            [8, 32, 256, 256]  f32
    conv_w       [192, 32, 3, 3]    f32   (192 = RANK(3) * C_OUT(64))
    kernel_weight[2, 256, 256]      f32
the reference:
    y   = conv2d(x, conv_w, stride 1, pad 1)      # [8, 192, 256, 256]
    y   = y.reshape(8, 3, 64, 256, 256)
    out = y[:,0] + kw[0]*y[:,1] + kw[1]*y[:,2]    # [8, 64, 256, 256]

Sharding: spatial (H) bands of 32 output rows across the 8 cores, all
batches per core, so the per-pixel blend weights are loaded once per core.

Per core, per supertile (4 output rows = 2 blocks q of 512 px):
  - conv matmuls in bf16 (FWL-able weight loads; rel err 3.4e-3 vs the
    2e-2 gate), K=96 (3 kh x 32 c_in), kw via free-dim shifts of the
    imcol tile (kh replicas pre-built host-side so each batch is ONE
    dma_start):
      per kw: bc[q0] (M=128: ranks 1|2), bc[q1], and a col-tiled
      concurrent pair a2[0:64] / a2[64:128] (M=64: rank 0, blocks q0/q1)
  - blend: m = bc * svb on DVE, then a col-tiled pair of identity
    matmuls folds m's two 64-row halves into a2 (which holds rank-0)
    -> out = y0 + sv1*y1 + sv2*y2 in PSUM; ACT evacuates to SBUF.
  - blend stage runs one supertile behind the conv stage so the PE
    never waits for DVE; a2/ident pairs are emitted as one adjacent
    pair-block per supertile (minimizes after-pair drain stalls).
  - DMA: per-batch imcol prefetched mid-previous-batch; head loads are
    ordered by deadline across both HWDGE rings (each dma_start costs
    ~700ns of sequencer time and data FIFOs behind earlier loads).
  - A ~10us dense warm-up burst of dummy matmuls at kernel start keeps
    the PE clock-gate (HAM) released: the real stream never un-throttles
    it by itself, and any >3us PE bubble re-throttles it permanently
    (2-4x slowdown). The burst outlasts every head DMA.
"""

import numpy as np
import ml_dtypes

B, C_IN, C_OUT, RANK, IMG = 8, 32, 64, 3, 256
N_CORES = 8
BAND = IMG // N_CORES          # 32 output rows per core
WP = IMG + 2                   # padded width 258
ROWS_IN = BAND + 2             # input rows needed per band (with halo)
SUPER = 8                      # supertiles per (batch, band): 4 rows each
SROWS = BAND // SUPER          # 4 image rows per supertile
NBLK = 512                     # pixels per matmul block (2 image rows)
IMW = BAND * WP                # imcol free size per kh block (8256)

_F32 = np.float32
_BF16 = ml_dtypes.bfloat16

# packed const layout (free-dim offsets in the [128, 640] wpack tile)
_OFF_WTBC = 0            # [0:96, kw*128 : kw*128+128], kw = 0..2
_OFF_WTA = 384           # [0:96, 384+kw*64 : +64]
_OFF_ID = 576            # [0:128, 576:640] = [I64; I64]
_WPACK_W = 640


def _build_bass():
    import concourse.mybir as mybir
    import concourse.tile as tile
    from concourse import bacc

    f32 = mybir.dt.float32
    bf16 = mybir.dt.bfloat16
    nc = bacc.Bacc("TRN2", target_bir_lowering=False, debug=False)

    # xs: host-prepped imcol source, partition dim (kh, c) pre-replicated
    xs_t = nc.dram_tensor("xs", (B, 96, IMW), bf16, kind="ExternalInput")
    wpack_t = nc.dram_tensor("wpack", (128, _WPACK_W), bf16, kind="ExternalInput")
    # svb rows 0:64 = sv1 (bcast over c), rows 64:128 = sv2; cols = (t, q, j)
    svb_t = nc.dram_tensor("svb", (128, SUPER * 2 * NBLK), bf16, kind="ExternalInput")
    out_t = nc.dram_tensor("out", (B, C_OUT, BAND, IMG), f32, kind="ExternalOutput")

    out_r = out_t.ap().rearrange(
        "b c (t q r) w -> b q c t (r w)", t=SUPER, q=2, r=SROWS // 2
    )

    with tile.TileContext(nc) as tc:
        with (
            tc.tile_pool(name="const", bufs=1) as cpool,
            tc.tile_pool(name="imcol", bufs=3) as ipool,
            tc.tile_pool(name="psum", bufs=2, space="PSUM") as ppool,
            tc.tile_pool(name="warm", bufs=1, space="PSUM") as wpool,
            tc.tile_pool(name="tmp", bufs=3) as tpool,
            tc.tile_pool(name="outp", bufs=6) as opool,
        ):
            # HAM warm-up: ~7us of dense back-to-back matmuls on dummy
            # data while the first imcol DMA is in flight. Without this
            # the PE clock-gate (K=4/8, 1.2 GHz) takes ~90us to release
            # (or never does) and every matmul runs 2-4x slow.
            dummy = cpool.tile([128, NBLK], bf16)
            nc.vector.memset(dummy[:], 0)
            warm_ps = wpool.tile([128, NBLK], f32, tag="warm")
            for _ in range(44):
                nc.tensor.matmul(
                    warm_ps[:], dummy[:, 0:128], dummy[:],
                    start=True, stop=True, skip_group_check=True,
                )
            # order matters: imcol(0) gates the first matmul -> sync ring,
            # first. consts on the ACT ring. svb halves late (first needed
            # by blend(0), ~3us after the first matmul).
            # first chunk covers supertiles 0-1 (rows 0..7) so the first
            # real matmul isn't gated on the whole batch
            imcols = {}
            imc = ipool.tile([96, IMW], bf16, tag="imcol")
            cut = 8 * WP
            nc.sync.dma_start(imc[:, 0:cut], xs_t.ap()[0][:, 0:cut])
            imcols[0] = imc

            wtbc_sb = cpool.tile([96, 3, 128], bf16)
            nc.scalar.dma_start(
                wtbc_sb[:],
                wpack_t.ap()[0:96, _OFF_WTBC : _OFF_WTBC + 384].rearrange(
                    "p (k m) -> p k m", k=3
                ),
            )
            wta_sb = cpool.tile([96, 3, 64], bf16)
            nc.scalar.dma_start(
                wta_sb[:],
                wpack_t.ap()[0:96, _OFF_WTA : _OFF_WTA + 192].rearrange(
                    "p (k m) -> p k m", k=3
                ),
            )
            idt_sb = cpool.tile([128, 64], bf16)
            nc.scalar.dma_start(idt_sb[:], wpack_t.ap()[:, _OFF_ID : _OFF_ID + 64])

            # sync-ring order = deadline order: imcol rows 0-7 (supertile
            # 0-1), svb half 1 (blend 0-3), imcol rest (supertile 2+),
            # svb half 2. Any PE bubble >~3us re-throttles the clock gate
            # permanently, so every load must land with margin.
            svb_sb = cpool.tile([128, SUPER, 2 * NBLK], bf16)
            svb_r = svb_t.ap().rearrange("p (t j) -> p t j", t=SUPER)
            nc.sync.dma_start(svb_sb[:, 0:4, :], svb_r[:, 0:4, :])
            nc.sync.dma_start(imc[:, cut:], xs_t.ap()[0][:, cut:])
            nc.sync.dma_start(svb_sb[:, 4:8, :], svb_r[:, 4:8, :])

            def wtbc(kw):
                return wtbc_sb[:, kw, :]

            def wta(kw):
                return wta_sb[:, kw, :]

            id_sb = idt_sb[:]

            # Flat software pipeline over (b, t): conv stage at i, blend
            # stage at i-1 so the PE never waits on DVE.
            steps = [(b, t) for b in range(B) for t in range(SUPER)]
            conv_state = {}  # i -> (bc, a2)

            for i, (b, t) in enumerate(steps):
                if t == 2 and b + 1 < B:
                    # prefetch next batch's imcol mid-batch (keeps the
                    # head clear; needed ~5 supertiles later)
                    imc = ipool.tile([96, IMW], bf16, tag="imcol")
                    nc.sync.dma_start(imc[:], xs_t.ap()[b + 1])
                    imcols[b + 1] = imc

                imv = imcols[b].rearrange("p (h w) -> p h w", w=WP)

                # ---- conv stage for step i ----
                bc = ppool.tile([128, 2, NBLK], f32, tag="bc")
                a2 = ppool.tile([128, NBLK], f32, tag="a2")
                hl = SROWS * t
                for kw in range(3):
                    rhs0 = imv[:, hl : hl + 2, kw : kw + IMG]
                    rhs1 = imv[:, hl + 2 : hl + 4, kw : kw + IMG]
                    nc.tensor.matmul(
                        bc[:, 0, :], wtbc(kw), rhs0,
                        start=(kw == 0), stop=(kw == 2),
                    )
                    nc.tensor.matmul(
                        bc[:, 1, :], wtbc(kw), rhs1,
                        start=(kw == 0), stop=(kw == 2),
                    )
                # blend of the previous step between the bc group and the
                # a2 group: its ident pair lands adjacent to the a2 pairs
                # (pair-block), and m(i-1) has had conv(i)'s bc time to
                # finish on DVE.
                if i >= 1:
                    _blend(nc, tpool, opool, conv_state, i - 1, steps,
                           svb_sb, id_sb, out_r, f32)
                    del conv_state[i - 1]

                for kw in range(3):
                    rhs0 = imv[:, hl : hl + 2, kw : kw + IMG]
                    rhs1 = imv[:, hl + 2 : hl + 4, kw : kw + IMG]
                    # rank-0 for both blocks: col-tiled concurrent pair
                    nc.tensor.matmul(
                        a2[0:64, :], wta(kw), rhs0,
                        start=(kw == 0), stop=False, skip_group_check=True,
                    )
                    nc.tensor.matmul(
                        a2[64:128, :], wta(kw), rhs1,
                        start=(kw == 0), stop=False, skip_group_check=True,
                    )
                conv_state[i] = (bc, a2)

            _blend(nc, tpool, opool, conv_state, len(steps) - 1, steps,
                   svb_sb, id_sb, out_r, f32, split=True)

    nc.compile()
    return nc


def _blend(nc, tpool, opool, conv_state, i, steps, svb_sb, id_sb, out_r, f32,
           split=False):
    import concourse.mybir as mybir

    bf16 = mybir.dt.bfloat16
    b, t = steps[i]
    bc, a2 = conv_state[i]
    m = tpool.tile([128, 2, NBLK], bf16, tag="m")
    svq = svb_sb[:, t, :].rearrange("p (q j) -> p q j", q=2)
    if split:
        # tail latency: fold q0 while q1's multiply still runs
        for q in range(2):
            nc.vector.tensor_tensor(
                m[:, q, :], bc[:, q, :], svq[:, q, :], mybir.AluOpType.mult
            )
            nc.tensor.matmul(
                a2[64 * q : 64 * q + 64, :], id_sb, m[:, q, :],
                start=False, stop=(q == 1), skip_group_check=True,
            )
    else:
        nc.vector.tensor_tensor(m[:], bc[:], svq, mybir.AluOpType.mult)
        # fold m's halves into a2: col-tiled concurrent pair
        nc.tensor.matmul(
            a2[0:64, :], id_sb, m[:, 0, :],
            start=False, stop=False, skip_group_check=True,
        )
        nc.tensor.matmul(
            a2[64:128, :], id_sb, m[:, 1, :],
            start=False, stop=True, skip_group_check=True,
        )
    out_sb = opool.tile([128, NBLK], f32, tag="out_sb")
    nc.scalar.copy(out_sb[:], a2[:])
    # one store per HWDGE ring: halves per-ring DIRECT2D sequencer load
    nc.sync.dma_start(out_r[b, 0, :, t, :], out_sb[0:64, :])
    nc.scalar.dma_start(out_r[b, 1, :, t, :], out_sb[64:128, :])


_CACHE = {}


def _get_bass():
    if "nc" not in _CACHE:
        _CACHE["nc"] = _build_bass()
    return _CACHE["nc"]


def _prep_shards(x, conv_w, kernel_weight):
    x = np.asarray(x, dtype=_F32)
    conv_w = np.asarray(conv_w, dtype=_F32)
    kernel_weight = np.asarray(kernel_weight, dtype=_F32)

    x_pad = np.pad(x, ((0, 0), (0, 0), (1, 1), (1, 1)))
    # w[kh, c, kw, (r, m)] from conv_w[(r m), c, kh, kw]
    wt = conv_w.transpose(2, 1, 3, 0).reshape(96, 3, RANK * C_OUT)
    wpack = np.zeros((128, _WPACK_W), dtype=_F32)
    for kw in range(3):
        wpack[0:96, _OFF_WTBC + 128 * kw : _OFF_WTBC + 128 * (kw + 1)] = (
            wt[:, kw, C_OUT:]
        )
        wpack[0:96, _OFF_WTA + 64 * kw : _OFF_WTA + 64 * (kw + 1)] = wt[:, kw, :C_OUT]
    eye = np.eye(64, dtype=_F32)
    wpack[0:64, _OFF_ID : _OFF_ID + 64] = eye
    wpack[64:128, _OFF_ID : _OFF_ID + 64] = eye
    wpack = wpack.astype(_BF16)

    in_maps = []
    for i in range(N_CORES):
        h0 = BAND * i
        band_x = x_pad[:, :, h0 : h0 + ROWS_IN, :]          # [B, 32, 34, 258]
        # imcol source: partition (kh, c) = rows kh..kh+BAND-1 flattened
        shard = np.empty((B, 96, IMW), dtype=_F32)
        for kh in range(3):
            shard[:, 32 * kh : 32 * kh + 32, :] = band_x[
                :, :, kh : kh + BAND, :
            ].reshape(B, 32, IMW)
        shard = shard.astype(_BF16)

        band = kernel_weight[:, h0 : h0 + BAND, :]          # [2, 32, 256]
        # svb[64r+c, (t, q, j)] = band[r, row(t, q, j)]
        arr = band.reshape(2, SUPER, 2 * NBLK)              # [r, t, (q j)]
        svb = np.broadcast_to(
            arr[:, None, :, :], (2, C_OUT, SUPER, 2 * NBLK)
        ).reshape(128, SUPER * 2 * NBLK)
        svb = np.ascontiguousarray(svb).astype(_BF16)
        in_maps.append({"xs": shard, "wpack": wpack, "svb": svb})
    return in_maps


def run(inputs, trace=False):
    """Run the sharded bass kernel; returns (out_full, BassKernelResults)."""
    from concourse.bass_utils import run_bass_kernel_spmd

    in_maps = _prep_shards(**inputs)
    nc = _get_bass()
    res = run_bass_kernel_spmd(
        nc, in_maps, core_ids=list(range(N_CORES)), trace=trace
    )
    out = np.empty((B, C_OUT, IMG, IMG), dtype=_F32)
    for i in range(N_CORES):
        out[:, :, BAND * i : BAND * (i + 1), :] = res.results[i]["out"]
    return out, res


def kernel(x, conv_w, kernel_weight):
    out, _ = run({"x": x, "conv_w": conv_w, "kernel_weight": kernel_weight})
    return out
